# revision 4
# baseline (speedup 1.0000x reference)
"""Trainium2 Bass kernel for the GSC Vanilla SNN problem.

3-layer LIF spiking net, S=101 timesteps, B=2048 batch, data-parallel over
batch across 8 NeuronCores (256 rows per core).

Math (per layer, per step, spikingjelly LIF with tau=2, v_th=1, hard reset),
using the doubled-state convention w = 2*v:
    a_t   = 0.5*w_{t-1} + c_t          (c_t = matmul current incl. bias)
    spike = (a_t >= 2)
    zeta  = (a_t < 2) in {0,1}
    w_t   = a_t * zeta
The true spike s = 1 - zeta is folded into the next layer's weights:
    s @ W + b  ==  zeta @ (-W) + (b + colsum(W))
and the bias rides the matmul itself as extra contraction rows:
  - layer 1: x is augmented with two ones-rows (K=122), W1 gets hi/lo bias rows
  - layers 2/3 + readout: pad lanes h>=200 of zeta are identically 1, so
    weight rows 200/201 carry the hi/lo bf16 bias split; rows 202+ are zero.
The readout accumulates R = sum_t zeta3_t @ Wr_pad in PSUM; the affine
correction (br + colsum(Wr)) - R/S and log_softmax run on the host during
unsharding.

Device layout: hidden dim padded 200->256; all [256, b] tensors stored as
[128 partitions, 512] tiles (cols 0:256 = h 0:128, cols 256:512 = h 128:256).
Engines: PE does all matmuls (bf16, fp32 PSUM); layer 1's LIF add reads PSUM
directly on DVE; layers 2/3 PSUM banks are evacuated to bf16 SBUF by the
Scalar engine (ACT) so their adds run in the DVE 2x bf16 mode.
"""

import numpy as np
import ml_dtypes

S = 101
D = 120            # C*M input features
DA = 122           # augmented with 2 ones-rows for hi/lo bias
H = 200
HP = 256           # padded hidden
DOUT = 12
NCORES = 8
B_FULL = 2048
BC = B_FULL // NCORES   # 256 batch rows per core
TB = 8                  # x DMA block (timesteps per DMA)

# engine assignment: per layer, how c leaves PSUM ("direct" = DVE reads PSUM
# in the LIF add; "act" = ACT copies PSUM->bf16 SBUF first), and which engine
# runs the zeta compare ("dve" | "gpsimd").
CFG = {
    "evac": ("direct", "act", "act"),
    "op2": ("dve", "dve", "dve"),
}

_bf16 = ml_dtypes.bfloat16

_BUILD_CACHE = {}


def _build(s_steps, bc, tb, cfg=None):
    """Build + compile the Bass program for one core. Returns nc."""
    import concourse.bacc as bacc
    import concourse.mybir as mybir
    import concourse.tile as tile

    cfg = cfg or CFG
    dt = mybir.dt
    alu = mybir.AluOpType
    P = 128
    B2 = 2 * bc  # free size of merged [128, 2*bc] tiles

    nc = bacc.Bacc("TRN2", target_bir_lowering=False, debug=False)

    x_d = nc.dram_tensor("x", [DA, s_steps * bc], dt.bfloat16, kind="ExternalInput")
    w1_d = nc.dram_tensor("w1", [DA, HP], dt.bfloat16, kind="ExternalInput")
    w2_d = nc.dram_tensor("w2", [2, P, HP], dt.bfloat16, kind="ExternalInput")
    w3_d = nc.dram_tensor("w3", [2, P, HP], dt.bfloat16, kind="ExternalInput")
    wr_d = nc.dram_tensor("wr", [2, P, DOUT], dt.bfloat16, kind="ExternalInput")
    out_d = nc.dram_tensor("out", [DOUT, bc], dt.float32, kind="ExternalOutput")

    with tile.TileContext(nc) as tc:
        with (
            tc.tile_pool(name="const", bufs=1) as constp,
            tc.tile_pool(name="xp", bufs=3) as xp,
            tc.tile_pool(name="state", bufs=1) as statep,
            tc.tile_pool(name="work", bufs=3) as workp,
            tc.tile_pool(name="ps", bufs=2, space="PSUM") as psp,
            tc.tile_pool(name="psr", bufs=1, space="PSUM") as psrp,
        ):
            w1 = constp.tile([DA, HP], dt.bfloat16)
            nc.sync.dma_start(w1[:], w1_d[:])
            w2a = constp.tile([P, HP], dt.bfloat16)
            w2b = constp.tile([P, HP], dt.bfloat16)
            nc.sync.dma_start(w2a[:], w2_d[0])
            nc.sync.dma_start(w2b[:], w2_d[1])
            w3a = constp.tile([P, HP], dt.bfloat16)
            w3b = constp.tile([P, HP], dt.bfloat16)
            nc.sync.dma_start(w3a[:], w3_d[0])
            nc.sync.dma_start(w3b[:], w3_d[1])
            wra = constp.tile([P, DOUT], dt.bfloat16)
            wrb = constp.tile([P, DOUT], dt.bfloat16)
            nc.sync.dma_start(wra[:], wr_d[0])
            nc.sync.dma_start(wrb[:], wr_d[1])

            st = [statep.tile([P, B2], dt.bfloat16, name=f"st{i}") for i in range(3)]
            for t_ in st:
                nc.vector.memset(t_[:], 0.0)

            R = psrp.tile([DOUT, bc], dt.float32)

            w23 = [(w2a, w2b), (w3a, w3b)]
            xb = None
            for t in range(s_steps):
                if t % tb == 0:
                    ncols = min(tb, s_steps - t) * bc
                    xb = xp.tile([DA, tb * bc], dt.bfloat16, name="xb")
                    nc.sync.dma_start(
                        xb[:, 0:ncols], x_d[:, t * bc : t * bc + ncols]
                    )
                xt = xb[:, (t % tb) * bc : (t % tb + 1) * bc]

                zs = []
                for li in range(3):
                    c = psp.tile([P, B2], dt.float32, name=f"c{li + 1}")
                    if li == 0:
                        nc.tensor.matmul(
                            c[:, 0:bc], w1[:, 0:P], xt, start=True, stop=True
                        )
                        nc.tensor.matmul(
                            c[:, bc:B2], w1[:, P:HP], xt, start=True, stop=True
                        )
                    else:
                        zprev = zs[-1]
                        wka, wkb = w23[li - 1]
                        for m in range(2):
                            nc.tensor.matmul(
                                c[:, m * bc : (m + 1) * bc],
                                wka[:, m * P : (m + 1) * P],
                                zprev[:, 0:bc],
                                start=True,
                                stop=False,
                            )
                            nc.tensor.matmul(
                                c[:, m * bc : (m + 1) * bc],
                                wkb[:, m * P : (m + 1) * P],
                                zprev[:, bc:B2],
                                start=False,
                                stop=True,
                            )

                    a = workp.tile([P, B2], dt.bfloat16, name=f"a{li + 1}")
                    if cfg["evac"][li] == "act":
                        ch = workp.tile([P, B2], dt.bfloat16, name=f"ch{li + 1}")
                        nc.scalar.copy(ch[:], c[:])
                        nc.vector.scalar_tensor_tensor(
                            a[:], st[li][:], 0.5, ch[:], op0=alu.mult, op1=alu.add
                        )
                    else:
                        nc.vector.scalar_tensor_tensor(
                            a[:], st[li][:], 0.5, c[:], op0=alu.mult, op1=alu.add
                        )
                    z = workp.tile([P, B2], dt.bfloat16, name=f"z{li + 1}")
                    eng2 = nc.gpsimd if cfg["op2"][li] == "gpsimd" else nc.vector
                    eng2.tensor_scalar(z[:], a[:], 2.0, None, alu.is_lt)
                    nc.vector.scalar_tensor_tensor(
                        st[li][:], a[:], 2.0, a[:], op0=alu.is_lt, op1=alu.mult
                    )
                    zs.append(z)

                z3 = zs[-1]
                nc.tensor.matmul(
                    R[:], wra[:], z3[:, 0:bc],
                    start=(t == 0), stop=False, skip_group_check=True,
                )
                nc.tensor.matmul(
                    R[:], wrb[:], z3[:, bc:B2],
                    start=False, stop=(t == s_steps - 1), skip_group_check=True,
                )

            out_sb = workp.tile([DOUT, bc], dt.float32)
            nc.vector.tensor_copy(out_sb[:], R[:])
            nc.sync.dma_start(out_d[:], out_sb[:])

    nc.compile()
    return nc


def _get_nc(s_steps=S, bc=BC, tb=TB):
    key = (s_steps, bc, tb)
    if key not in _BUILD_CACHE:
        _BUILD_CACHE[key] = _build(s_steps, bc, tb)
    return _BUILD_CACHE[key]


def _hi_lo(v):
    hi = v.astype(_bf16)
    lo = (v - hi.astype(np.float64)).astype(_bf16)
    return hi, lo


def _prep_weights(W1, b1, W2, b2, W3, b3, Wr, br):
    """Host-side weight packing. Returns (device array dict, host affine base)."""
    P = 128

    def pad(w, rows, cols, neg=False):
        w = np.asarray(w, np.float64)
        if neg:
            w = -w
        out = np.zeros((rows, cols), np.float64)
        out[: w.shape[0], : w.shape[1]] = w
        return out

    w1p = pad(W1, DA, HP)
    bh = np.zeros(HP, np.float64)
    bh[:H] = np.asarray(b1, np.float64)
    w1p_bf = w1p.astype(_bf16)
    w1p_bf[D], w1p_bf[D + 1] = _hi_lo(bh)

    def mid(W, b):
        wp = pad(W, HP, HP, neg=True).astype(_bf16)
        bh = np.zeros(HP, np.float64)
        bh[:H] = np.asarray(b, np.float64) + np.asarray(W, np.float64).sum(axis=0)
        wp[H], wp[H + 1] = _hi_lo(bh)
        return wp.reshape(2, P, HP)

    w2p = mid(W2, b2)
    w3p = mid(W3, b3)
    wrp = pad(Wr, HP, DOUT).astype(_bf16).reshape(2, P, DOUT)

    base = (np.asarray(br, np.float64) + np.asarray(Wr, np.float64).sum(axis=0)).astype(
        np.float32
    )
    return {"w1": w1p_bf, "w2": w2p, "w3": w3p, "wr": wrp}, base


def _prep_x(x):
    """[B,C,S,M] f32 -> per-core [DA, S*bc] bf16 list (with two ones-rows)."""
    x = np.asarray(x, np.float32)
    B = x.shape[0]
    bc = B // NCORES
    # [C, M, S, B] -> [D, S, B]
    xt = np.ascontiguousarray(x.transpose(1, 3, 2, 0)).reshape(D, S, B).astype(_bf16)
    outs = []
    for i in range(NCORES):
        xc = np.ones((DA, S * bc), dtype=_bf16)
        xc[:D] = xt[:, :, i * bc : (i + 1) * bc].reshape(D, S * bc)
        outs.append(xc)
    return outs


def _postprocess(R_list, base):
    """R per core [12, bc] -> full [B, 12] log_softmax output."""
    outs = []
    for R in R_list:
        o = base[None, :] - R.T.astype(np.float32) / S
        m = o.max(axis=1, keepdims=True)
        z = o - m
        lse = np.log(np.exp(z).sum(axis=1, keepdims=True))
        outs.append(z - lse)
    return np.concatenate(outs, axis=0).astype(np.float32)


def _ensure_ntff_hook():
    """Inject antenv.axon_hooks (NTFF profile hook) if the image lacks it."""
    import sys
    try:
        from antenv.axon_hooks import get_axon_ntff_profile_hook  # noqa: F401
        return True
    except ImportError:
        pass
    import contextlib
    import ctypes
    import types

    so_path = "/opt/axon/libaxon_pjrt.so"
    try:
        lib = ctypes.CDLL(so_path)
    except OSError:
        return False
    if not hasattr(lib, "axon_start_nrt_profile"):
        return False
    lib.axon_start_nrt_profile.argtypes = [
        ctypes.POINTER(ctypes.c_int64),
        ctypes.c_size_t,
    ]
    lib.axon_start_nrt_profile.restype = ctypes.c_int64
    lib.axon_stop_nrt_profile.argtypes = [ctypes.c_char_p]
    lib.axon_stop_nrt_profile.restype = ctypes.c_int64

    @contextlib.contextmanager
    def _hook(output_dir, device_ids):
        import jax

        jax.devices()
        if device_ids:
            ids = (ctypes.c_int64 * len(device_ids))(*device_ids)
            rc = lib.axon_start_nrt_profile(ids, len(device_ids))
        else:
            rc = lib.axon_start_nrt_profile(None, 0)
        if rc != 0:
            raise RuntimeError(f"axon_start_nrt_profile rc={rc}")
        try:
            yield
        finally:
            n = lib.axon_stop_nrt_profile(str(output_dir).encode())
            if n < 0:
                raise RuntimeError(f"axon_stop_nrt_profile rc={n}")

    mod = types.ModuleType("antenv.axon_hooks")
    mod._hook = _hook
    mod.get_axon_ntff_profile_hook = lambda: _hook
    mod.set_axon_ntff_profile_hook = lambda h: setattr(mod, "_hook", h)
    import antenv

    sys.modules["antenv.axon_hooks"] = mod
    antenv.axon_hooks = mod
    return True


def kernel(x, W1, b1, W2, b2, W3, b3, Wr, br, _trace=False):
    from concourse.bass_utils import run_bass_kernel_spmd

    if _trace:
        _trace = _ensure_ntff_hook()

    nc = _get_nc()
    wmap, base = _prep_weights(W1, b1, W2, b2, W3, b3, Wr, br)
    xs = _prep_x(x)
    in_maps = [{**wmap, "x": xs[i]} for i in range(NCORES)]
    res = run_bass_kernel_spmd(
        nc, in_maps, core_ids=list(range(NCORES)), trace=_trace
    )
    R_list = [res.results[i]["out"] for i in range(NCORES)]
    out = _postprocess(R_list, base)
    if _trace:
        kernel.last_exec_time_ns = res.exec_time_ns
        kernel.last_results = res
    return out


kernel.last_exec_time_ns = None
kernel.last_results = None
